# revision 1
# baseline (speedup 1.0000x reference)
# Neural-collapse regularizer (tr_SW / tr_SB) on 8 TRN2 NeuronCores.
#
# Math: with per-class sums S_c = sum_{i: l_i=c} x_i, counts n_c,
# ssq = sum_i ||x_i||^2:
#   tr_SW = ssq - sum_c ||S_c||^2 / n_c
#   tr_SB = sum_c ||S_c/n_c - g||^2,  g = (sum_c S_c) / N
# So the device only needs the segment sums [C, D] and ssq; everything
# else is tiny O(C*D) host math.
#
# Sharding: class-parallel. Core k owns classes [128k, 128(k+1)); the host
# routes each row to the core that owns its label (segment sum is
# order-invariant so any within-core row order is fine).
#
# Layout trick: rows are packed in chunks of GRP=8 rows of a single class,
# one chunk per (group, partition) slot. All 8 row-tiles of a group then
# share one [128x128] one-hot (built once per group on DVE) and one
# stationary operand for all the group's matmuls.
#
# ssq trick: each row's sum-of-squares s[p] is produced by a fused
# square+row-reduce (DVE scalar_tensor_tensor or ACT Square+accum_out,
# split across both engines for balance) directly into a bf16 "s slot"
# at column 512 of the row's 514-wide lane. The second matmul of each
# tile covers columns [256, 513): its 257th output column accumulates
# per-class sum-of-squares in PSUM for free.

import contextlib
import ctypes
import os
import sys
import types

import numpy as np
import ml_dtypes

import concourse.bass as bass
import concourse.bacc as bacc
import concourse.mybir as mybir
import concourse.tile as tile
from concourse.bass_utils import run_bass_kernel_spmd


def _ensure_ntff_hook():
    """Provide antenv.axon_hooks + an NTFF profile hook when the image's
    antenv package lacks it (needed only for trace=True timing runs)."""
    try:
        from antenv.axon_hooks import get_axon_ntff_profile_hook  # noqa: F401
        return
    except ImportError:
        pass
    mod = types.ModuleType("antenv.axon_hooks")
    state = {"hook": None}
    mod.set_axon_ntff_profile_hook = lambda h: state.__setitem__("hook", h)
    mod.get_axon_ntff_profile_hook = lambda: state["hook"]
    sys.modules["antenv.axon_hooks"] = mod

    so_path = "/opt/axon/libaxon_pjrt.so"
    if not os.path.exists(so_path):
        return
    lib = ctypes.CDLL(so_path)
    if not hasattr(lib, "axon_start_nrt_profile"):
        return
    lib.axon_start_nrt_profile.argtypes = [
        ctypes.POINTER(ctypes.c_int64), ctypes.c_size_t]
    lib.axon_start_nrt_profile.restype = ctypes.c_int64
    lib.axon_stop_nrt_profile.argtypes = [ctypes.c_char_p]
    lib.axon_stop_nrt_profile.restype = ctypes.c_int64

    @contextlib.contextmanager
    def _hook(output_dir, device_ids):
        import jax
        jax.devices()
        if device_ids:
            ids = (ctypes.c_int64 * len(device_ids))(*device_ids)
            rc = lib.axon_start_nrt_profile(ids, len(device_ids))
        else:
            rc = lib.axon_start_nrt_profile(None, 0)
        if rc != 0:
            raise RuntimeError(f"axon_start_nrt_profile rc={rc}")
        try:
            yield
        finally:
            n = lib.axon_stop_nrt_profile(str(output_dir).encode())
            print(f"profile: {n} file(s) written to {output_dir}",
                  file=sys.stderr)

    mod.set_axon_ntff_profile_hook(_hook)


CORES = 8
P = 128              # partitions = classes per core
D = 512              # feature dim (asserted against input)
GRP = 8              # row-tiles per group = rows per chunk
LANE = D + 2         # per-tile lane: 512 features, 1 s-slot, 1 pad (align)
HALF = D // 2
BF16 = mybir.dt.bfloat16
F32 = mybir.dt.float32
NP_BF16 = ml_dtypes.bfloat16

# Per-tile engine split for the sum-of-squares work (weights, any scale):
#   A: DVE scalar_tensor_tensor (square + row-reduce, 1x, ~604ns)
#   B: ACT Square + accum_out (~805ns)
#   C: DVE tensor_tensor mult (2x, ~327ns) + PE ones-matmul reduce (~240ns)
W_A = float(os.environ.get("K_W_A", "37"))
W_B = float(os.environ.get("K_W_B", "55"))
W_C = float(os.environ.get("K_W_C", "44"))


def _host_shard(features: np.ndarray, labels: np.ndarray):
    """Chunked class-sorted layout.

    Returns (in_maps, G). in_maps[k]:
      feat: [G, 128, GRP*LANE] bf16 -- slot (g, p) holds GRP rows of one
            class at j*LANE offsets; cols 512/513 of each lane are zero.
      lab:  [128, G] f32 -- rebased class (0..127) of slot (g, p)
      iota: [128, 128] bf16
    """
    N, d = features.shape
    assert d == D, f"expected D={D}, got {d}"
    CPAD = CORES * P

    order = np.argsort(labels, kind="stable")
    sl = labels[order]
    class_start = np.searchsorted(sl, np.arange(CPAD + 1))  # [1025]
    counts = np.diff(class_start)                            # [1024]
    chunks_per_class = -(-counts // GRP)                     # ceil
    core_chunks = chunks_per_class.reshape(CORES, P)
    G = int(-(-core_chunks.sum(axis=1).max() // P))

    fbf = features.astype(NP_BF16)
    iota = np.broadcast_to(np.arange(P, dtype=NP_BF16), (P, P)).copy()

    in_maps = []
    for k in range(CORES):
        nch = core_chunks[k]                    # chunks per rebased class
        total = int(nch.sum())
        assert total <= G * P
        # chunk m -> class: repeat
        chunk_cls = np.repeat(np.arange(P), nch)             # [total]
        # padded row grid: [G*P, GRP] of global row indices, -1 = empty
        grid = np.full((G * P, GRP), -1, dtype=np.int64)
        # scatter each class's rows into its chunks
        cls_pad_start = np.concatenate(([0], np.cumsum(nch * GRP)))  # [129]
        cnts = counts[k * P:(k + 1) * P]
        lo = class_start[k * P]
        n_k = int(cnts.sum())
        rows_k = order[lo:lo + n_k]
        lab_k = sl[lo:lo + n_k] - k * P          # rebased, sorted 0..127
        within = np.arange(n_k) - np.repeat(class_start[k * P:(k + 1) * P] - lo,
                                            cnts)
        pos = np.repeat(cls_pad_start[:-1], cnts) + within
        grid.reshape(-1)[pos] = rows_k

        # gather features; zero the padding rows
        safe = np.maximum(grid, 0)
        fr = fbf[safe.reshape(-1)]               # [G*P*GRP, D]
        fr[grid.reshape(-1) < 0] = 0
        fr = fr.reshape(G * P, GRP, D)

        feat = np.zeros((G * P, GRP, LANE), dtype=NP_BF16)
        feat[:, :, :D] = fr
        # chunk m -> (g = m // P, p = m % P)
        feat = feat.reshape(G, P, GRP * LANE)

        labg = np.zeros((G * P,), dtype=np.float32)
        labg[:total] = chunk_cls
        labg = np.ascontiguousarray(labg.reshape(G, P).T)    # [128, G]

        in_maps.append({"feat": feat, "lab": labg, "iota": iota})
    return in_maps, G



# per-group tile pattern: first N_A tiles -> DVE STT (fused square+row-sum),
# next N_B -> ACT Square+accum_out, last N_C -> DVE 2x TT-mult + PE
# ones-matmul reduce. Balances DVE/ACT/PE under the ~49us DMA floor.
N_A = int(os.environ.get("K_N_A", "2"))
N_B = int(os.environ.get("K_N_B", "3"))
N_C = GRP - N_A - N_B

XB = int(os.environ.get("K_XB", "8"))    # xg group buffers
OHB = 4                                   # one-hot buffers
SQCB = 3                                  # scratch rotation depth (groups)

def _build_raw(G: int):
    T = G * GRP
    nc = bacc.Bacc("TRN2", target_bir_lowering=False, debug=False,
                   enable_asserts=False)
    feat_h = nc.dram_tensor("feat", [G, P, GRP * LANE], BF16,
                            kind="ExternalInput")
    lab_h = nc.dram_tensor("lab", [P, G], F32, kind="ExternalInput")
    iota_h = nc.dram_tensor("iota", [P, P], BF16, kind="ExternalInput")
    out_h = nc.dram_tensor("out", [P, D + 2], F32, kind="ExternalOutput")

    x_sb = nc.alloc_sbuf_tensor("x_sb", [P, XB * GRP * LANE], BF16)
    oh_sb = nc.alloc_sbuf_tensor("oh_sb", [P, OHB * P], BF16)
    sqd_sb = nc.alloc_sbuf_tensor("sqd_sb", [P, SQCB * N_A * D], BF16)
    sqa_sb = nc.alloc_sbuf_tensor("sqa_sb", [P, SQCB * N_B * D], BF16)
    sqc_sb = nc.alloc_sbuf_tensor("sqc_sb", [P, SQCB * N_C * D], BF16)
    iota_sb = nc.alloc_sbuf_tensor("iota_sb", [P, P], BF16)
    lab_sb = nc.alloc_sbuf_tensor("lab_sb", [P, G], F32)
    ones_sb = nc.alloc_sbuf_tensor("ones_sb", [P, 1], BF16)
    out_sb = nc.alloc_sbuf_tensor("out_sb", [P, D + 2], F32)
    psum_a = nc.alloc_psum_tensor("psum_a", [P, D], F32)
    psum_b = nc.alloc_psum_tensor("psum_b", [P, D], F32)
    psum_c = nc.alloc_psum_tensor("psum_c", [P, D], F32)

    xg_ap = lambda g: x_sb.ap()[:, (g % XB) * GRP * LANE:
                                (g % XB + 1) * GRP * LANE]
    oh_ap = lambda g: oh_sb.ap()[:, (g % OHB) * P:(g % OHB + 1) * P]

    import contextlib as _ctx
    with (
        _ctx.ExitStack() as _sems,
                nc.semaphore("sem_oh") as sem_oh,
        nc.semaphore("sem_sd") as sem_sd,
        nc.semaphore("sem_sa") as sem_sa,
        nc.semaphore("sem_pe") as sem_pe,
        nc.semaphore("sem_cp") as sem_cp,
        nc.semaphore("sem_out") as sem_out,
        nc.semaphore("sem_ones") as sem_ones,
        nc.semaphore("sem_iota") as sem_iota,
        nc.semaphore("sem_lab") as sem_lab,
        nc.Block() as block,
    ):
        sem_xs = [_sems.enter_context(nc.semaphore(f"sem_x{b}"))
                  for b in range(XB)]

        def wait_x(eng, g):
            eng.wait_ge(sem_xs[g % XB], 16 * (g // XB + 1))
        @block.gpsimd
        def _(gpsimd):
            gpsimd.memset(ones_sb.ap(), 1.0)
            gpsimd.memset(out_sb.ap()[:, D + 1:D + 2], 0.0).then_inc(
                sem_ones, 1)


        @block.sync
        def _(sync):
            sync.dma_start(out=xg_ap(0), in_=feat_h.ap()[0]).then_inc(
                sem_xs[0], 16)
            sync.dma_start(out=iota_sb.ap(), in_=iota_h.ap()).then_inc(
                sem_iota, 16)
            sync.dma_start(out=lab_sb.ap(), in_=lab_h.ap()).then_inc(
                sem_lab, 16)
            for g in range(1, G):
                if g >= XB:
                    sync.wait_ge(sem_pe, g - XB + 1)
                sync.dma_start(out=xg_ap(g), in_=feat_h.ap()[g]).then_inc(
                    sem_xs[g % XB], 16)
            sync.wait_ge(sem_cp, 1)
            sync.dma_start(out=out_h.ap(), in_=out_sb.ap()).then_inc(
                sem_out, 16)
            sync.wait_ge(sem_out, 16)

        @block.vector
        def _(vector):
            vector.wait_ge(sem_iota, 16)
            vector.wait_ge(sem_lab, 16)
            with nc.allow_low_precision("bf16 row sums; aggregate err ~1e-5"):
                for g in range(G):
                    wait_x(vector, g)
                    if g >= SQCB:
                        vector.wait_ge(sem_pe, g - SQCB + 1)
                    xg = xg_ap(g)
                    vector.tensor_scalar(
                        oh_ap(g), iota_sb.ap(), lab_sb.ap()[:, g:g + 1], None,
                        mybir.AluOpType.is_equal,
                    ).then_inc(sem_oh, 1)
                    last = None
                    for j in range(N_A):
                        off = j * LANE
                        dbuf = (g % SQCB) * N_A + j
                        last = vector.scalar_tensor_tensor(
                            out=sqd_sb.ap()[:, dbuf * D:(dbuf + 1) * D],
                            in0=xg[:, off:off + D], scalar=1.0,
                            in1=xg[:, off:off + D],
                            op0=mybir.AluOpType.mult,
                            op1=mybir.AluOpType.mult,
                            accum_out=xg[:, off + D:off + D + 1],
                        )
                    for i in range(N_C):
                        j = N_A + N_B + i
                        off = j * LANE
                        buf = (g % SQCB) * N_C + i
                        last = vector.tensor_tensor(
                            out=sqc_sb.ap()[:, buf * D:(buf + 1) * D],
                            in0=xg[:, off:off + D], in1=xg[:, off:off + D],
                            op=mybir.AluOpType.mult,
                        )
                    assert last is not None
                    last.then_inc(sem_sd, 1)
                # tail: copy psums out
                vector.wait_ge(sem_pe, G)
                vector.wait_ge(sem_ones, 1)
                vector.tensor_copy(out=out_sb.ap()[:, 0:HALF],
                                   in_=psum_a.ap()[:, 0:HALF])
                vector.tensor_copy(out=out_sb.ap()[:, HALF:D + 1],
                                   in_=psum_b.ap()[:, 0:HALF + 1])
                vector.tensor_reduce(
                    out=out_sb.ap()[0:1, D + 1:D + 2],
                    in_=psum_c.ap()[0:1, :],
                    axis=mybir.AxisListType.X, op=mybir.AluOpType.add,
                ).then_inc(sem_cp, 1)

        @block.scalar
        def _(scalar):
            with nc.allow_low_precision("bf16 row sums; aggregate err ~1e-5"):
                for g in range(G):
                    wait_x(scalar, g)
                    if g >= SQCB:
                        scalar.wait_ge(sem_pe, g - SQCB + 1)
                    xg = xg_ap(g)
                    last = None
                    for i in range(N_B):
                        j = N_A + i
                        off = j * LANE
                        abuf = (g % SQCB) * N_B + i
                        last = scalar.activation(
                            sqa_sb.ap()[:, abuf * D:(abuf + 1) * D],
                            xg[:, off:off + D],
                            mybir.ActivationFunctionType.Square,
                            accum_out=xg[:, off + D:off + D + 1],
                        )
                    last.then_inc(sem_sa, 1)

        @block.tensor
        def _(tensor):
            tensor.wait_ge(sem_ones, 1)
            for g in range(G):
                t0 = g * GRP
                tensor.wait_ge(sem_oh, g + 1)
                wait_x(tensor, g)
                xg = xg_ap(g)
                oh = oh_ap(g)
                for j in range(GRP):
                    off = j * LANE
                    tensor.matmul(
                        out=psum_a.ap()[:, 0:HALF], lhsT=oh,
                        rhs=xg[:, off:off + HALF],
                        start=(t0 + j == 0), stop=(t0 + j == T - 1),
                    )
                tensor.wait_ge(sem_sd, g + 1)
                tensor.wait_ge(sem_sa, g + 1)
                last = None
                for j in range(GRP):
                    off = j * LANE
                    last = tensor.matmul(
                        out=psum_b.ap()[:, 0:HALF + 1], lhsT=oh,
                        rhs=xg[:, off + HALF:off + D + 1],
                        start=(t0 + j == 0), stop=(t0 + j == T - 1),
                    )
                for i in range(N_C):
                    buf = (g % SQCB) * N_C + i
                    last = tensor.matmul(
                        out=psum_c.ap()[0:1, :], lhsT=ones_sb.ap(),
                        rhs=sqc_sb.ap()[:, buf * D:(buf + 1) * D],
                        start=(g == 0 and i == 0),
                        stop=(g == G - 1 and i == N_C - 1),
                    )
                last.then_inc(sem_pe, 1)

    nc.compile()
    return nc



def _finalize(results, labels: np.ndarray, C: int, N: int):
    sums = np.concatenate(
        [np.asarray(r["out"][:, :D], dtype=np.float64) for r in results], axis=0
    )  # [1024, D]
    ssq = float(sum(np.asarray(r["out"][:, D], dtype=np.float64).sum()
                    + float(r["out"][0, D + 1])
                    for r in results))
    counts = np.bincount(labels, minlength=CORES * P).astype(np.float64)

    sums = sums[:C]
    counts = counts[:C]
    means = sums / counts[:, None]
    g = sums.sum(axis=0) / N
    tr_sw = ssq - float(((sums * sums).sum(axis=1) / counts).sum())
    tr_sb = float(((means - g) ** 2).sum())
    return np.asarray(np.float32(tr_sw / tr_sb))


def run(features, labels, num_classes, trace=False):
    features = np.asarray(features, dtype=np.float32)
    labels = np.asarray(labels).astype(np.int64).ravel()
    C = int(num_classes)
    N = features.shape[0]
    assert C <= CORES * P, f"num_classes={C} exceeds {CORES * P}"

    if trace:
        _ensure_ntff_hook()
    in_maps, G = _host_shard(features, labels)
    nc = _build_raw(G)
    res = run_bass_kernel_spmd(nc, in_maps, list(range(CORES)), trace=trace)
    out = _finalize(res.results, labels, C, N)
    return out, res


def kernel(**inputs) -> np.ndarray:
    trace = os.environ.get("KERNEL_TRACE", "0") == "1"
    out, _ = run(inputs["features"], inputs["labels"], inputs["num_classes"],
                 trace=trace)
    return out

